# revision 33
# baseline (speedup 1.0000x reference)
"""Segment-masked attention kernel for Trainium2 (8 NeuronCores).

Problem: B=1, H=16, S=4096, D=128, NSEG=2 segment-id masked softmax attention.

Strategy:
  * Host: stable-argsort q positions and kv positions by segment id. With the
    rows/cols grouped by segment, the masked attention decomposes into two
    dense block-diagonal attentions (one per segment) -- no per-element mask
    work on device and ~half the FLOPs. Outputs are scattered back to the
    original q order on host. (Softmax over kv is permutation invariant, so
    permuting kv changes nothing; permuting q is undone at the end.)
    Segments are padded to even sizes (fp32r ISA needs even free dims); kv
    padding uses zero k/v rows whose exp(0)=1 is subtracted from the sums.
  * Shard: 2 heads per core across 8 cores (head-parallel, no comms).
  * All matmuls use dt.float32r: 1 cycle/column on the PE (4x faster than
    fp32) at ~1.5e-4 relative precision (tf32-like).
  * Device (per head, per segment): q blocks (<=512 cols) processed in pairs;
    for each kv chunk of 128 rows:
        sT[kv,q]   = matmul(lhsT=kT_chunk, rhs=qT_block)        (PE)
        pT[kv,q]   = exp(scale * sT)   (ACT, one call per pair, PSUM src)
        oT[d,q]   += matmul(lhsT=v_chunk, rhs=pT)               (PE, accum)
        sums[2,q] += matmul(lhsT=ones[:,2], rhs=pT)             (PE, accum)
    oT and sums stream to DRAM; the host divides by sums and transposes back.
    No max-subtraction is needed: scaled scores are ~N(0,1); exp never
    overflows fp32 and softmax is shift invariant.
"""

import os

import numpy as np

_PROGRAM_CACHE = {}
last_exec_time_ns = None


def _install_ntff_hook():
    """Provide antenv.axon_hooks (missing in this image) so that
    run_bass_kernel_spmd(trace=True) can capture an NTFF profile."""
    import contextlib
    import ctypes
    import sys
    import types

    try:
        from antenv.axon_hooks import get_axon_ntff_profile_hook  # noqa: F401

        return True  # real module exists
    except ImportError:
        pass

    so_path = "/opt/axon/libaxon_pjrt.so"
    if not os.path.exists(so_path):
        return False
    lib = ctypes.CDLL(so_path)
    if not hasattr(lib, "axon_start_nrt_profile"):
        return False
    lib.axon_start_nrt_profile.argtypes = [
        ctypes.POINTER(ctypes.c_int64),
        ctypes.c_size_t,
    ]
    lib.axon_start_nrt_profile.restype = ctypes.c_int64
    lib.axon_stop_nrt_profile.argtypes = [ctypes.c_char_p]
    lib.axon_stop_nrt_profile.restype = ctypes.c_int64

    @contextlib.contextmanager
    def _hook(output_dir, device_ids):
        import jax

        jax.devices()
        if device_ids:
            ids = (ctypes.c_int64 * len(device_ids))(*device_ids)
            rc = lib.axon_start_nrt_profile(ids, len(device_ids))
        else:
            rc = lib.axon_start_nrt_profile(None, 0)
        if rc != 0:
            raise RuntimeError(f"axon_start_nrt_profile rc={rc}")
        try:
            yield
        finally:
            n = lib.axon_stop_nrt_profile(str(output_dir).encode())
            print(f"ntff profile: {n} file(s) written to {output_dir}")

    holder = [_hook]
    mod = types.ModuleType("antenv.axon_hooks")
    mod.set_axon_ntff_profile_hook = lambda h: holder.__setitem__(0, h)
    mod.get_axon_ntff_profile_hook = lambda: holder[0]
    sys.modules["antenv.axon_hooks"] = mod
    import antenv

    antenv.axon_hooks = mod
    return True


def _build_program(S, D, hpc, mq, nk, kv_padded):
    """mq/nk: per-segment q/kv sizes AFTER host padding (all even).
    kv_padded[g]: segment g's kv range ends with one zero dummy row, whose
    exp(0)=1 contribution must be subtracted from the softmax sums.
    Outputs O^T [hpc, D, Sq] and softmax sums [hpc, Sq]; the host divides
    and transposes back."""
    import concourse.bacc as bacc
    import concourse.mybir as mybir
    import concourse.tile as tile

    f32 = mybir.dt.float32
    f32r = mybir.dt.float32r
    Exp = mybir.ActivationFunctionType.Exp
    scale = 1.0 / float(np.sqrt(D))

    Sq = sum(mq)
    Skv = sum(nk)

    nc = bacc.Bacc("TRN2", target_bir_lowering=False, debug=False)

    qT_d = nc.dram_tensor("qT", [hpc, D, Sq], f32r, kind="ExternalInput")
    kT_d = nc.dram_tensor("kT", [hpc, D, Skv], f32r, kind="ExternalInput")
    v_d = nc.dram_tensor("v", [hpc, Skv, D], f32r, kind="ExternalInput")
    o_d = nc.dram_tensor("o", [hpc, D, Sq], f32, kind="ExternalOutput")
    sums_d = nc.dram_tensor("sums", [hpc, Sq], f32, kind="ExternalOutput")

    # segment ranges after the host-side sort+pad
    seg_q = [(0, mq[0]), (mq[0], mq[0] + mq[1])]
    seg_kv = [(0, nk[0]), (nk[0], nk[0] + nk[1])]

    QB = 512  # q block width (moving free dim); processed in pairs
    KC = 128  # kv chunk (contraction rows per matmul)

    with tile.TileContext(nc) as tc:
        ctxs = []

        def pool(**kw):
            p = tc.tile_pool(**kw)
            ctxs.append(p)
            return p.__enter__()

        singles = pool(name="singles", bufs=1)
        pt_pool = pool(name="pt", bufs=6)
        otsb_pool = pool(name="otsb", bufs=4)
        sums_sb_pool = pool(name="sums_sb", bufs=4)
        psum_s = pool(name="psum_s", bufs=2, space="PSUM")
        psum_ot = pool(name="psum_ot", bufs=1, space="PSUM")
        psum_sums = pool(name="psum_sums", bufs=1, space="PSUM")

        ones_stage = singles.tile([128, 2], f32)
        nc.vector.memset(ones_stage, 1.0)
        ones_col = singles.tile([128, 2], f32r)
        nc.vector.tensor_copy(ones_col, ones_stage)

        # ---- input loads (critical pieces for head 0 / segment 0 first) ----
        qT_sb = {}
        kT_sb = {}
        v_sb = {}  # (head, seg) -> [128, C_g, 128] tile, kv rows packed per seg
        for h in range(hpc):
            qT_sb[h] = singles.tile([128, Sq], f32r, tag=f"qT{h}", name=f"qT_sb{h}")
            kT_sb[h] = singles.tile([128, Skv], f32r, tag=f"kT{h}", name=f"kT_sb{h}")
            for g, (kv0, kv1) in enumerate(seg_kv):
                L = kv1 - kv0
                if L <= 0:
                    continue
                C = (L + KC - 1) // KC
                v_sb[(h, g)] = singles.tile(
                    [128, C, 128], f32r, tag=f"v{h}_{g}", name=f"v_sb{h}_{g}"
                )

        def load_head(h, first):
            # ordered by first use; for the very first head the leading pieces
            # are small so compute can start within ~2us
            pieces = [256, 768, 1024] if first else [1024]
            q0e = 0
            for p in pieces:
                pe = min(q0e + p, Sq)
                if pe > q0e:
                    nc.sync.dma_start(
                        out=qT_sb[h][:, q0e:pe], in_=qT_d[h, :, q0e:pe]
                    )
                q0e = pe
            for g, (kv0, kv1) in enumerate(seg_kv):
                L = kv1 - kv0
                if L <= 0:
                    continue
                po = kv0
                for p in [256, 768, 1024, 2048] if (first and g == 0) else []:
                    pe = min(po + p, kv1)
                    if pe > po:
                        nc.sync.dma_start(
                            out=kT_sb[h][:, po:pe], in_=kT_d[h, :, po:pe]
                        )
                    po = pe
                while po < kv1:
                    pe = min(po + 2048, kv1)
                    nc.sync.dma_start(
                        out=kT_sb[h][:, po:pe], in_=kT_d[h, :, po:pe]
                    )
                    po = pe
                vt = v_sb[(h, g)]
                nfull = L // KC
                csplits = (
                    sorted({min(x, nfull) for x in (0, 4, 16, nfull)})
                    if (first and g == 0)
                    else [0, nfull]
                )
                for c0, c1 in zip(csplits, csplits[1:]):
                    if c1 <= c0:
                        continue
                    src = v_d[
                        h, kv0 + c0 * KC : kv0 + c1 * KC, :
                    ].rearrange("(c p) d -> p c d", p=KC)
                    nc.sync.dma_start(out=vt[:, c0:c1, :], in_=src)
                rtail = L - nfull * KC
                if rtail:
                    nc.sync.dma_start(
                        out=vt[:rtail, nfull, :],
                        in_=v_d[h, kv0 + nfull * KC : kv1, :],
                    )
            while q0e < Sq:
                pe = min(q0e + 2048, Sq)
                nc.sync.dma_start(out=qT_sb[h][:, q0e:pe], in_=qT_d[h, :, q0e:pe])
                q0e = pe

        for h in range(hpc):
            load_head(h, first=(h == 0))

        # ---- main compute ----
        for h in range(hpc):
            for g, (q0g, q1g) in enumerate(seg_q):
                kv0, kv1 = seg_kv[g]
                if q1g <= q0g or kv1 <= kv0:
                    continue
                chunks = [(ck, min(KC, kv1 - ck)) for ck in range(kv0, kv1, KC)]
                C = len(chunks)
                block_starts = list(range(q0g, q1g, QB))
                pairs = [block_starts[i : i + 2] for i in range(0, len(block_starts), 2)]
                for pair in pairs:
                    blocks = [(qo, min(QB, q1g - qo)) for qo in pair]
                    nb = len(blocks)
                    uniform = nb == 2 and blocks[0][1] == blocks[1][1]
                    ot_ps = psum_ot.tile([128, 2, QB], f32, tag="ot")
                    sums_ps = psum_sums.tile([2, 2, QB], f32, tag="sums")
                    pts = [None] * C
                    # software pipeline: scores/exp run 2 chunks ahead of pv/sums
                    for j in range(C + 2):
                        if j < C:
                            ck, cw = chunks[j]
                            s_ps = psum_s.tile([128, 2, QB], f32, tag="s")
                            for b, (qo, W) in enumerate(blocks):
                                nc.tensor.matmul(
                                    s_ps[:cw, b, :W],
                                    kT_sb[h][:, ck : ck + cw],
                                    qT_sb[h][:, qo : qo + W],
                                    start=True,
                                    stop=True,
                                )
                            pt = pt_pool.tile([128, 2, QB], f32r, tag="pt", name="pt")
                            pts[j] = pt
                            if uniform:
                                W = blocks[0][1]
                                nc.scalar.activation(
                                    pt[:cw, :, :W], s_ps[:cw, :, :W], Exp, scale=scale
                                )
                            else:
                                for b, (qo, W) in enumerate(blocks):
                                    nc.scalar.activation(
                                        pt[:cw, b, :W], s_ps[:cw, b, :W], Exp, scale=scale
                                    )
                        if j >= 2:
                            jj = j - 2
                            ck, cw = chunks[jj]
                            pt = pts[jj]
                            for b, (qo, W) in enumerate(blocks):
                                nc.tensor.matmul(
                                    ot_ps[:, b, :W],
                                    v_sb[(h, g)][:cw, jj, :],
                                    pt[:cw, b, :W],
                                    start=(jj == 0),
                                    stop=(jj == C - 1),
                                )
                            for b, (qo, W) in enumerate(blocks):
                                nc.tensor.matmul(
                                    sums_ps[:2, b, :W],
                                    ones_col[:cw, :],
                                    pt[:cw, b, :W],
                                    start=(jj == 0),
                                    stop=(jj == C - 1),
                                )
                    # epilogue: copy O^T and corrected sums to SBUF, DMA out
                    for b, (qo, W) in enumerate(blocks):
                        ot_sb = otsb_pool.tile([128, QB], f32, tag="otsb")
                        nc.vector.tensor_copy(ot_sb[:, :W], ot_ps[:, b, :W])
                        nc.sync.dma_start(
                            out=o_d[h, :, qo : qo + W], in_=ot_sb[:, :W]
                        )
                        sums_sb = sums_sb_pool.tile([1, QB], f32, tag="sums_sb")
                        # kv dummy row (k=0) contributed exp(0)=1 to every sum
                        nc.vector.tensor_scalar_add(
                            sums_sb[:1, :W],
                            sums_ps[:1, b, :W],
                            -1.0 if kv_padded[g] else 0.0,
                        )
                        nc.sync.dma_start(
                            out=sums_d[h : h + 1, qo : qo + W],
                            in_=sums_sb[:1, :W],
                        )

        for p in reversed(ctxs):
            p.__exit__(None, None, None)

    nc.compile()
    return nc


def kernel(q, k, v, q_segment_ids, kv_segment_ids):
    global last_exec_time_ns
    from concourse.bass_utils import run_bass_kernel_spmd

    q = np.asarray(q, dtype=np.float32)
    k = np.asarray(k, dtype=np.float32)
    v = np.asarray(v, dtype=np.float32)
    q_seg = np.asarray(q_segment_ids, dtype=np.int32)
    kv_seg = np.asarray(kv_segment_ids, dtype=np.int32)

    B, H, S, D = q.shape
    assert B == 1
    ncores = 8
    hpc = H // ncores

    qperm = np.argsort(q_seg[0], kind="stable")
    kvperm = np.argsort(kv_seg[0], kind="stable")
    m0 = int((q_seg[0] == 0).sum())
    n0 = int((kv_seg[0] == 0).sum())
    m1, n1 = S - m0, S - n0

    # fp32r matmuls need even free sizes -> pad every segment to even length
    # (q dummies: computed but never stored; kv dummies: k=0,v=0 rows whose
    # exp(0)=1 is subtracted from the softmax sums on device)
    def pad_seg(arr_s, lens):
        # arr_s: [H, S, D] sorted; split into segments, pad each to even
        parts, out_lens = [], []
        off = 0
        for L in lens:
            seg = arr_s[:, off : off + L, :]
            if L % 2:
                z = np.zeros((arr_s.shape[0], 1, arr_s.shape[2]), arr_s.dtype)
                seg = np.concatenate([seg, z], axis=1)
            parts.append(seg)
            out_lens.append(seg.shape[1])
            off += L
        return np.concatenate(parts, axis=1), out_lens

    q_s, mq = pad_seg(q[0][:, qperm, :], [m0, m1])
    k_s, nk = pad_seg(k[0][:, kvperm, :], [n0, n1])
    v_s, _ = pad_seg(v[0][:, kvperm, :], [n0, n1])
    kv_padded = (n0 % 2 == 1, n1 % 2 == 1)
    qT = np.ascontiguousarray(np.swapaxes(q_s, 1, 2))  # [H, D, Sq]
    kT = np.ascontiguousarray(np.swapaxes(k_s, 1, 2))

    key = (S, D, hpc, tuple(mq), tuple(nk), kv_padded)
    if key not in _PROGRAM_CACHE:
        _PROGRAM_CACHE.clear()
        _PROGRAM_CACHE[key] = _build_program(S, D, hpc, mq, nk, kv_padded)
    nc = _PROGRAM_CACHE[key]

    in_maps = []
    for i in range(ncores):
        hs = slice(i * hpc, (i + 1) * hpc)
        in_maps.append(
            {
                "qT": np.ascontiguousarray(qT[hs]),
                "kT": np.ascontiguousarray(kT[hs]),
                "v": np.ascontiguousarray(v_s[hs]),
            }
        )

    trace = bool(int(os.environ.get("KERNEL_TRACE", "0")))
    tmpdir = None
    if trace:
        trace = _install_ntff_hook()
        tmpdir = os.environ.get("KERNEL_TRACE_DIR") or None
        if trace:
            import concourse.bass_utils as _bu

            _bu.upload_artifacts = lambda d: d  # no bucket access here
    res = run_bass_kernel_spmd(
        nc, in_maps, core_ids=list(range(ncores)), trace=trace, tmpdir=tmpdir
    )
    last_exec_time_ns = res.exec_time_ns

    oT_pad = np.concatenate([res.results[i]["o"] for i in range(ncores)], axis=0)
    sums_pad = np.concatenate(
        [res.results[i]["sums"] for i in range(ncores)], axis=0
    )
    # normalize (device returns unnormalized O^T and softmax sums),
    # transpose back to [H, Sq, D]
    o_pad = np.swapaxes(oT_pad / sums_pad[:, None, :], 1, 2)
    # drop q dummy rows (end of each padded segment), then unsort
    o_sorted = np.concatenate([o_pad[:, :m0, :], o_pad[:, mq[0] : mq[0] + m1, :]], 1)
    out = np.empty((H, S, D), dtype=np.float32)
    out[:, qperm, :] = o_sorted
    return np.ascontiguousarray(out[None], dtype=np.float32)


# revision 34
# speedup vs baseline: 1.0294x; 1.0294x over previous
"""Segment-masked attention kernel for Trainium2 (8 NeuronCores).

Problem: B=1, H=16, S=4096, D=128, NSEG=2 segment-id masked softmax attention.

Strategy:
  * Host: stable-argsort q positions and kv positions by segment id. With the
    rows/cols grouped by segment, the masked attention decomposes into two
    dense block-diagonal attentions (one per segment) -- no per-element mask
    work on device and ~half the FLOPs. Outputs are scattered back to the
    original q order on host. (Softmax over kv is permutation invariant, so
    permuting kv changes nothing; permuting q is undone at the end.)
    Segments are padded to even sizes (fp32r ISA needs even free dims); kv
    padding uses zero k/v rows whose exp(0)=1 is subtracted from the sums.
  * Shard: 2 heads per core across 8 cores (head-parallel, no comms).
  * All matmuls use dt.float32r: 1 cycle/column on the PE (4x faster than
    fp32) at ~1.5e-4 relative precision (tf32-like).
  * Device (per head, per segment): q blocks (<=512 cols) processed in pairs;
    for each kv chunk of 128 rows:
        sT[kv,q]   = matmul(lhsT=kT_chunk, rhs=qT_block)        (PE)
        pT[kv,q]   = exp(scale * sT)   (ACT, one call per pair, PSUM src)
        oT[d,q]   += matmul(lhsT=v_chunk, rhs=pT)               (PE, accum)
        sums[2,q] += matmul(lhsT=ones[:,2], rhs=pT)             (PE, accum)
    oT and sums stream to DRAM; the host divides by sums and transposes back.
    No max-subtraction is needed: scaled scores are ~N(0,1); exp never
    overflows fp32 and softmax is shift invariant.
"""

import os

import numpy as np

_PROGRAM_CACHE = {}
last_exec_time_ns = None


def _install_ntff_hook():
    """Provide antenv.axon_hooks (missing in this image) so that
    run_bass_kernel_spmd(trace=True) can capture an NTFF profile."""
    import contextlib
    import ctypes
    import sys
    import types

    try:
        from antenv.axon_hooks import get_axon_ntff_profile_hook  # noqa: F401

        return True  # real module exists
    except ImportError:
        pass

    so_path = "/opt/axon/libaxon_pjrt.so"
    if not os.path.exists(so_path):
        return False
    lib = ctypes.CDLL(so_path)
    if not hasattr(lib, "axon_start_nrt_profile"):
        return False
    lib.axon_start_nrt_profile.argtypes = [
        ctypes.POINTER(ctypes.c_int64),
        ctypes.c_size_t,
    ]
    lib.axon_start_nrt_profile.restype = ctypes.c_int64
    lib.axon_stop_nrt_profile.argtypes = [ctypes.c_char_p]
    lib.axon_stop_nrt_profile.restype = ctypes.c_int64

    @contextlib.contextmanager
    def _hook(output_dir, device_ids):
        import jax

        jax.devices()
        if device_ids:
            ids = (ctypes.c_int64 * len(device_ids))(*device_ids)
            rc = lib.axon_start_nrt_profile(ids, len(device_ids))
        else:
            rc = lib.axon_start_nrt_profile(None, 0)
        if rc != 0:
            raise RuntimeError(f"axon_start_nrt_profile rc={rc}")
        try:
            yield
        finally:
            n = lib.axon_stop_nrt_profile(str(output_dir).encode())
            print(f"ntff profile: {n} file(s) written to {output_dir}")

    holder = [_hook]
    mod = types.ModuleType("antenv.axon_hooks")
    mod.set_axon_ntff_profile_hook = lambda h: holder.__setitem__(0, h)
    mod.get_axon_ntff_profile_hook = lambda: holder[0]
    sys.modules["antenv.axon_hooks"] = mod
    import antenv

    antenv.axon_hooks = mod
    return True


def _build_program(S, D, hpc, mq, nk, kv_padded):
    """mq/nk: per-segment q/kv sizes AFTER host padding (all even).
    kv_padded[g]: segment g's kv range ends with one zero dummy row, whose
    exp(0)=1 contribution must be subtracted from the softmax sums.
    Outputs O^T [hpc, D, Sq] and softmax sums [hpc, Sq]; the host divides
    and transposes back."""
    import concourse.bacc as bacc
    import concourse.mybir as mybir
    import concourse.tile as tile

    f32 = mybir.dt.float32
    f32r = mybir.dt.float32r
    Exp = mybir.ActivationFunctionType.Exp
    scale = 1.0 / float(np.sqrt(D))

    Sq = sum(mq)
    Skv = sum(nk)

    nc = bacc.Bacc("TRN2", target_bir_lowering=False, debug=False)

    qT_d = nc.dram_tensor("qT", [hpc, D, Sq], f32r, kind="ExternalInput")
    kT_d = nc.dram_tensor("kT", [hpc, D, Skv], f32r, kind="ExternalInput")
    v_d = nc.dram_tensor("v", [hpc, Skv, D], f32r, kind="ExternalInput")
    o_d = nc.dram_tensor("o", [hpc, D, Sq], f32, kind="ExternalOutput")
    sums_d = nc.dram_tensor("sums", [hpc, Sq], f32, kind="ExternalOutput")

    # segment ranges after the host-side sort+pad
    seg_q = [(0, mq[0]), (mq[0], mq[0] + mq[1])]
    seg_kv = [(0, nk[0]), (nk[0], nk[0] + nk[1])]

    QB = 512  # q block width (moving free dim); processed in pairs
    KC = 128  # kv chunk (contraction rows per matmul)

    with tile.TileContext(nc) as tc:
        ctxs = []

        def pool(**kw):
            p = tc.tile_pool(**kw)
            ctxs.append(p)
            return p.__enter__()

        singles = pool(name="singles", bufs=1)
        pt_pool = pool(name="pt", bufs=6)
        otsb_pool = pool(name="otsb", bufs=4)
        sums_sb_pool = pool(name="sums_sb", bufs=4)
        psum_s = pool(name="psum_s", bufs=2, space="PSUM")
        psum_ot = pool(name="psum_ot", bufs=1, space="PSUM")
        psum_sums = pool(name="psum_sums", bufs=1, space="PSUM")

        ones_stage = singles.tile([128, 2], f32)
        nc.vector.memset(ones_stage, 1.0)
        ones_col = singles.tile([128, 2], f32r)
        nc.vector.tensor_copy(ones_col, ones_stage)

        # ---- input loads (critical pieces for head 0 / segment 0 first) ----
        qT_sb = {}
        kT_sb = {}
        v_sb = {}  # (head, seg) -> [128, C_g, 128] tile, kv rows packed per seg
        for h in range(hpc):
            qT_sb[h] = singles.tile([128, Sq], f32r, tag=f"qT{h}", name=f"qT_sb{h}")
            kT_sb[h] = singles.tile([128, Skv], f32r, tag=f"kT{h}", name=f"kT_sb{h}")
            for g, (kv0, kv1) in enumerate(seg_kv):
                L = kv1 - kv0
                if L <= 0:
                    continue
                C = (L + KC - 1) // KC
                v_sb[(h, g)] = singles.tile(
                    [128, C, 128], f32r, tag=f"v{h}_{g}", name=f"v_sb{h}_{g}"
                )

        def load_head(h, first):
            # ordered by first use; for the very first head the leading pieces
            # are small so compute can start within ~2us
            pieces = [256, 768, 1024] if first else [1024]
            q0e = 0
            for p in pieces:
                pe = min(q0e + p, Sq)
                if pe > q0e:
                    nc.sync.dma_start(
                        out=qT_sb[h][:, q0e:pe], in_=qT_d[h, :, q0e:pe]
                    )
                q0e = pe
            for g, (kv0, kv1) in enumerate(seg_kv):
                L = kv1 - kv0
                if L <= 0:
                    continue
                po = kv0
                for p in [256, 768, 1024, 2048] if (first and g == 0) else []:
                    pe = min(po + p, kv1)
                    if pe > po:
                        nc.sync.dma_start(
                            out=kT_sb[h][:, po:pe], in_=kT_d[h, :, po:pe]
                        )
                    po = pe
                while po < kv1:
                    pe = min(po + 2048, kv1)
                    nc.sync.dma_start(
                        out=kT_sb[h][:, po:pe], in_=kT_d[h, :, po:pe]
                    )
                    po = pe
                vt = v_sb[(h, g)]
                nfull = L // KC
                csplits = (
                    sorted({min(x, nfull) for x in (0, 4, 16, nfull)})
                    if (first and g == 0)
                    else [0, nfull]
                )
                for c0, c1 in zip(csplits, csplits[1:]):
                    if c1 <= c0:
                        continue
                    src = v_d[
                        h, kv0 + c0 * KC : kv0 + c1 * KC, :
                    ].rearrange("(c p) d -> p c d", p=KC)
                    nc.sync.dma_start(out=vt[:, c0:c1, :], in_=src)
                rtail = L - nfull * KC
                if rtail:
                    nc.sync.dma_start(
                        out=vt[:rtail, nfull, :],
                        in_=v_d[h, kv0 + nfull * KC : kv1, :],
                    )
            while q0e < Sq:
                pe = min(q0e + 2048, Sq)
                nc.sync.dma_start(out=qT_sb[h][:, q0e:pe], in_=qT_d[h, :, q0e:pe])
                q0e = pe

        for h in range(hpc):
            load_head(h, first=(h == 0))

        # ---- main compute ----
        for h in range(hpc):
            for g, (q0g, q1g) in enumerate(seg_q):
                kv0, kv1 = seg_kv[g]
                if q1g <= q0g or kv1 <= kv0:
                    continue
                chunks = [(ck, min(KC, kv1 - ck)) for ck in range(kv0, kv1, KC)]
                C = len(chunks)
                block_starts = list(range(q0g, q1g, QB))
                # a narrow partial block must not run as a singleton pair: it
                # would pay every chunk's LDWEIGHTS against a tiny matmul
                # stream. Pair it with a full block so the loads hide.
                if (
                    len(block_starts) % 2 == 1
                    and len(block_starts) >= 3
                    and q1g - block_starts[-1] < QB
                ):
                    block_starts = (
                        [block_starts[0], block_starts[-1]] + block_starts[1:-1]
                    )
                pairs = [block_starts[i : i + 2] for i in range(0, len(block_starts), 2)]
                for pair in pairs:
                    blocks = [(qo, min(QB, q1g - qo)) for qo in pair]
                    nb = len(blocks)
                    uniform = nb == 2 and blocks[0][1] == blocks[1][1]
                    ot_ps = psum_ot.tile([128, 2, QB], f32, tag="ot")
                    sums_ps = psum_sums.tile([2, 2, QB], f32, tag="sums")
                    pts = [None] * C
                    # software pipeline: scores/exp run 2 chunks ahead of pv/sums
                    for j in range(C + 2):
                        if j < C:
                            ck, cw = chunks[j]
                            s_ps = psum_s.tile([128, 2, QB], f32, tag="s")
                            for b, (qo, W) in enumerate(blocks):
                                nc.tensor.matmul(
                                    s_ps[:cw, b, :W],
                                    kT_sb[h][:, ck : ck + cw],
                                    qT_sb[h][:, qo : qo + W],
                                    start=True,
                                    stop=True,
                                )
                            pt = pt_pool.tile([128, 2, QB], f32r, tag="pt", name="pt")
                            pts[j] = pt
                            if uniform:
                                W = blocks[0][1]
                                nc.scalar.activation(
                                    pt[:cw, :, :W], s_ps[:cw, :, :W], Exp, scale=scale
                                )
                            else:
                                for b, (qo, W) in enumerate(blocks):
                                    nc.scalar.activation(
                                        pt[:cw, b, :W], s_ps[:cw, b, :W], Exp, scale=scale
                                    )
                        if j >= 2:
                            jj = j - 2
                            ck, cw = chunks[jj]
                            pt = pts[jj]
                            for b, (qo, W) in enumerate(blocks):
                                nc.tensor.matmul(
                                    ot_ps[:, b, :W],
                                    v_sb[(h, g)][:cw, jj, :],
                                    pt[:cw, b, :W],
                                    start=(jj == 0),
                                    stop=(jj == C - 1),
                                )
                            for b, (qo, W) in enumerate(blocks):
                                nc.tensor.matmul(
                                    sums_ps[:2, b, :W],
                                    ones_col[:cw, :],
                                    pt[:cw, b, :W],
                                    start=(jj == 0),
                                    stop=(jj == C - 1),
                                )
                    # epilogue: copy O^T and corrected sums to SBUF, DMA out
                    for b, (qo, W) in enumerate(blocks):
                        ot_sb = otsb_pool.tile([128, QB], f32, tag="otsb")
                        nc.vector.tensor_copy(ot_sb[:, :W], ot_ps[:, b, :W])
                        nc.sync.dma_start(
                            out=o_d[h, :, qo : qo + W], in_=ot_sb[:, :W]
                        )
                        sums_sb = sums_sb_pool.tile([1, QB], f32, tag="sums_sb")
                        # kv dummy row (k=0) contributed exp(0)=1 to every sum
                        nc.vector.tensor_scalar_add(
                            sums_sb[:1, :W],
                            sums_ps[:1, b, :W],
                            -1.0 if kv_padded[g] else 0.0,
                        )
                        nc.sync.dma_start(
                            out=sums_d[h : h + 1, qo : qo + W],
                            in_=sums_sb[:1, :W],
                        )

        for p in reversed(ctxs):
            p.__exit__(None, None, None)

    nc.compile()
    return nc


def kernel(q, k, v, q_segment_ids, kv_segment_ids):
    global last_exec_time_ns
    from concourse.bass_utils import run_bass_kernel_spmd

    q = np.asarray(q, dtype=np.float32)
    k = np.asarray(k, dtype=np.float32)
    v = np.asarray(v, dtype=np.float32)
    q_seg = np.asarray(q_segment_ids, dtype=np.int32)
    kv_seg = np.asarray(kv_segment_ids, dtype=np.int32)

    B, H, S, D = q.shape
    assert B == 1
    ncores = 8
    hpc = H // ncores

    qperm = np.argsort(q_seg[0], kind="stable")
    kvperm = np.argsort(kv_seg[0], kind="stable")
    m0 = int((q_seg[0] == 0).sum())
    n0 = int((kv_seg[0] == 0).sum())
    m1, n1 = S - m0, S - n0

    # fp32r matmuls need even free sizes -> pad every segment to even length
    # (q dummies: computed but never stored; kv dummies: k=0,v=0 rows whose
    # exp(0)=1 is subtracted from the softmax sums on device)
    def pad_seg(arr_s, lens):
        # arr_s: [H, S, D] sorted; split into segments, pad each to even
        parts, out_lens = [], []
        off = 0
        for L in lens:
            seg = arr_s[:, off : off + L, :]
            if L % 2:
                z = np.zeros((arr_s.shape[0], 1, arr_s.shape[2]), arr_s.dtype)
                seg = np.concatenate([seg, z], axis=1)
            parts.append(seg)
            out_lens.append(seg.shape[1])
            off += L
        return np.concatenate(parts, axis=1), out_lens

    q_s, mq = pad_seg(q[0][:, qperm, :], [m0, m1])
    k_s, nk = pad_seg(k[0][:, kvperm, :], [n0, n1])
    v_s, _ = pad_seg(v[0][:, kvperm, :], [n0, n1])
    kv_padded = (n0 % 2 == 1, n1 % 2 == 1)
    qT = np.ascontiguousarray(np.swapaxes(q_s, 1, 2))  # [H, D, Sq]
    kT = np.ascontiguousarray(np.swapaxes(k_s, 1, 2))

    key = (S, D, hpc, tuple(mq), tuple(nk), kv_padded)
    if key not in _PROGRAM_CACHE:
        _PROGRAM_CACHE.clear()
        _PROGRAM_CACHE[key] = _build_program(S, D, hpc, mq, nk, kv_padded)
    nc = _PROGRAM_CACHE[key]

    in_maps = []
    for i in range(ncores):
        hs = slice(i * hpc, (i + 1) * hpc)
        in_maps.append(
            {
                "qT": np.ascontiguousarray(qT[hs]),
                "kT": np.ascontiguousarray(kT[hs]),
                "v": np.ascontiguousarray(v_s[hs]),
            }
        )

    trace = bool(int(os.environ.get("KERNEL_TRACE", "0")))
    tmpdir = None
    if trace:
        trace = _install_ntff_hook()
        tmpdir = os.environ.get("KERNEL_TRACE_DIR") or None
        if trace:
            import concourse.bass_utils as _bu

            _bu.upload_artifacts = lambda d: d  # no bucket access here
    res = run_bass_kernel_spmd(
        nc, in_maps, core_ids=list(range(ncores)), trace=trace, tmpdir=tmpdir
    )
    last_exec_time_ns = res.exec_time_ns

    oT_pad = np.concatenate([res.results[i]["o"] for i in range(ncores)], axis=0)
    sums_pad = np.concatenate(
        [res.results[i]["sums"] for i in range(ncores)], axis=0
    )
    # normalize (device returns unnormalized O^T and softmax sums),
    # transpose back to [H, Sq, D]
    o_pad = np.swapaxes(oT_pad / sums_pad[:, None, :], 1, 2)
    # drop q dummy rows (end of each padded segment), then unsort
    o_sorted = np.concatenate([o_pad[:, :m0, :], o_pad[:, mq[0] : mq[0] + m1, :]], 1)
    out = np.empty((H, S, D), dtype=np.float32)
    out[:, qperm, :] = o_sorted
    return np.ascontiguousarray(out[None], dtype=np.float32)
